# revision 1
# baseline (speedup 1.0000x reference)
"""CTLSTM cell fused kernel for 8 Trainium2 NeuronCores.

Strategy (data-parallel over batch):
  - B=16384 rows sharded 2048/core; weights replicated.
  - Host stages transposed operands so the K contraction dim lands on SBUF
    partitions: xh = [x;ht].T -> [1024, 2048/core], w2 = [Wx;Wh].T ->
    [1024, 3584], both cast to bf16 (PE runs 1 col/cycle and FWL hides the
    weight loads; fp32 would serialize a ~190ns LDWEIGHTS per matmul).
    PSUM accumulation stays fp32.
  - Gate columns are host-permuted to [z, d, i, f, o, i_bar, f_bar] so the
    five sigmoid gates are contiguous: per 128-row subtile ACT runs one
    tanh, one sigmoid(-x) and ONE [128,2560] sigmoid, all in place in a
    contiguous [128,3584] pre-activation mega-tile.
  - bf16 allows N=1024 moving: matmuls compute gate PAIRS into 2-bank
    PSUM tiles; DVE drains each pair with a single fused bias-add.
  - softplus(wd) has no ACT table set; computed as -ln(sigmoid(-wd)).
    sigmoid(-wd) from the main pass is stashed in SBUF; Ln chunks at the
    end are forced (explicit deps) after all main-pass ACT ops so the
    activation table switches exactly once.
"""

import numpy as np
import ml_dtypes

import concourse.bacc as bacc
import concourse.bass as bass
import concourse.mybir as mybir
import concourse.tile as tile
from concourse.tile_rust import add_dep_helper
from concourse.bass_utils import run_bass_kernel_spmd

NCORES = 8
B = 16384
I = 512
H = 512
NG = 7
G = NG * H          # 3584
K2 = I + H          # 1024
P = 128
BS = B // NCORES    # 2048 rows per core
NT = BS // P        # 16 subtiles of 128 rows
SUP = 4             # subtiles per supertile (DMA granularity)
NSUP = NT // SUP

BF16 = mybir.dt.bfloat16
F32 = mybir.dt.float32
AF = mybir.ActivationFunctionType
NPBF16 = ml_dtypes.bfloat16

# gate order in the permuted weight/bias layout (reference order is
# i, f, z, o, d, i_bar, f_bar)
PERM = [2, 4, 0, 1, 3, 5, 6]   # -> z, d, i, f, o, i_bar, f_bar

TRACE = False
LAST_RESULTS = None

_nc_cache = None


def _build():
    nc = bacc.Bacc("TRN2", target_bir_lowering=False, debug=False)

    xh = nc.dram_tensor("xh", [K2, BS], BF16, kind="ExternalInput")
    w2 = nc.dram_tensor("w2", [K2, G], BF16, kind="ExternalInput")
    ct = nc.dram_tensor("ct", [BS, H], F32, kind="ExternalInput")
    bb_d = nc.dram_tensor("bb", [P, G], F32, kind="ExternalInput")

    h_d = nc.dram_tensor("h", [BS, H], F32, kind="ExternalOutput")
    c_d = nc.dram_tensor("c", [BS, H], F32, kind="ExternalOutput")
    cb_d = nc.dram_tensor("cb", [BS, H], F32, kind="ExternalOutput")
    o_d = nc.dram_tensor("o", [BS, H], F32, kind="ExternalOutput")
    dr_d = nc.dram_tensor("dr", [BS, H], F32, kind="ExternalOutput")

    last_sn = None  # final main-pass ACT instruction, gates phase 2

    with tile.TileContext(nc) as tc:
        with (
            tc.tile_pool(name="wp", bufs=1) as wp,
            tc.tile_pool(name="cp", bufs=1) as cp,
            tc.tile_pool(name="sp", bufs=1) as sp,
            tc.tile_pool(name="xp", bufs=2) as xp,
            tc.tile_pool(name="ctp", bufs=4) as ctp,
            tc.tile_pool(name="gp", bufs=2) as gp,
            tc.tile_pool(name="pp", bufs=3, space=bass.MemorySpace.PSUM) as pp,
            tc.tile_pool(name="pps", bufs=2, space=bass.MemorySpace.PSUM) as pps,
        ):
            # resident weights: 8 K-chunks of [128, 3584] bf16
            w_sb = []
            for k in range(8):
                wt = wp.tile([P, G], BF16, tag=f"w{k}")
                nc.sync.dma_start(wt[:], w2[k * P:(k + 1) * P, :])
                w_sb.append(wt)
            # broadcast bias [128, 3584] fp32 (bx+bh, host-staged broadcast)
            bb = cp.tile([P, G], F32, tag="bb")
            nc.sync.dma_start(bb[:], bb_d[:])
            # sigmoid(-wd) stash, one [128, 512] slice per subtile
            stash = sp.tile([P, NT, H], F32, tag="stash")

            for s in range(NSUP):
                xhs = []
                for k in range(8):
                    t_ = xp.tile([P, SUP * P], BF16, tag=f"xh{k}")
                    nc.sync.dma_start(
                        t_[:], xh[k * P:(k + 1) * P, s * SUP * P:(s + 1) * SUP * P]
                    )
                    xhs.append(t_)

                for j in range(SUP):
                    t = s * SUP + j
                    bsl = slice(j * P, (j + 1) * P)
                    rows = slice(t * P, (t + 1) * P)

                    ctj = ctp.tile([P, H], F32, tag="ct")
                    nc.sync.dma_start(ctj[:], ct[rows, :])

                    ga = gp.tile([P, G], F32, tag="ga")

                    # gate pairs (z,d), (i,f), (o,ib) then single (fb); each
                    # pair accumulates in a 2-bank PSUM tile drained by one
                    # fused bias-add
                    for pr in range(3):
                        csl = slice(pr * 2 * H, (pr + 1) * 2 * H)
                        acc = pp.tile([P, 2 * H], F32, tag="accp")
                        for half in range(2):
                            gsl = slice((pr * 2 + half) * H,
                                        (pr * 2 + half + 1) * H)
                            hsl = slice(half * H, (half + 1) * H)
                            for k in range(8):
                                nc.tensor.matmul(
                                    acc[:, hsl], xhs[k][:, bsl], w_sb[k][:, gsl],
                                    start=(k == 0), stop=(k == 7),
                                )
                        nc.vector.tensor_add(ga[:, csl], acc[:], bb[:, csl])
                    csl = slice(6 * H, 7 * H)
                    acc = pps.tile([P, H], F32, tag="accs")
                    for k in range(8):
                        nc.tensor.matmul(
                            acc[:], xhs[k][:, bsl], w_sb[k][:, csl],
                            start=(k == 0), stop=(k == 7),
                        )
                    nc.vector.tensor_add(ga[:, csl], acc[:], bb[:, csl])

                    # permuted gate slices of ga
                    Z = ga[:, 0 * H:1 * H]
                    D = ga[:, 1 * H:2 * H]
                    Ii = ga[:, 2 * H:3 * H]
                    F = ga[:, 3 * H:4 * H]
                    O = ga[:, 4 * H:5 * H]
                    IB = ga[:, 5 * H:6 * H]
                    FB = ga[:, 6 * H:7 * H]

                    nc.scalar.activation(Z, Z, AF.Tanh)
                    nc.scalar.activation(stash[:, t, :], D, AF.Sigmoid,
                                         scale=-1.0)
                    nc.scalar.activation(ga[:, 2 * H:], ga[:, 2 * H:], AF.Sigmoid)

                    nc.sync.dma_start(o_d[rows, :], O)

                    nc.vector.tensor_mul(F, F, ctj[:])    # f*ct
                    nc.vector.tensor_mul(Ii, Ii, Z)       # i*z
                    nc.vector.tensor_add(F, F, Ii)        # c
                    nc.sync.dma_start(c_d[rows, :], F)
                    nc.vector.tensor_mul(IB, IB, Z)       # ib*z
                    last_sn = nc.scalar.activation(Z, F, AF.Tanh)  # tanh(c)
                    nc.vector.tensor_mul(FB, FB, ctj[:])  # fb*ct
                    nc.vector.tensor_add(FB, FB, IB)      # cbar
                    nc.sync.dma_start(cb_d[rows, :], FB)
                    nc.vector.tensor_mul(Z, O, Z)         # h = o*tanh(c)
                    nc.sync.dma_start(h_d[rows, :], Z)

            # phase 2: decay_rate = softplus(wd) = -ln(sigmoid(-wd))
            dr_r = dr_d.rearrange("(n t p) c -> n p t c", t=SUP, p=P)
            for chn in range(NSUP):
                chsl = slice(chn * SUP, (chn + 1) * SUP)
                ln = nc.scalar.activation(stash[:, chsl, :], stash[:, chsl, :],
                                          AF.Ln)
                # keep Ln after every main-pass ACT: one table switch total
                add_dep_helper(ln.ins, last_sn.ins, reason="phase2 after phase1")
                nc.vector.tensor_scalar_mul(stash[:, chsl, :], stash[:, chsl, :],
                                            -1.0)
                nc.sync.dma_start(dr_r[chn], stash[:, chsl, :])

    nc.compile()
    return nc




def kernel(x, ht, ct, Wx, bx, Wh, bh):
    global _nc_cache, LAST_RESULTS
    if _nc_cache is None:
        _nc_cache = _build()
    nc = _nc_cache

    x = np.ascontiguousarray(x, dtype=np.float32)
    ht = np.ascontiguousarray(ht, dtype=np.float32)
    ct = np.ascontiguousarray(ct, dtype=np.float32)

    # host staging: transpose/concat/cast + gate permutation + bias broadcast
    xh_full = np.empty((K2, B), dtype=NPBF16)
    xh_full[:I, :] = x.T.astype(NPBF16)
    xh_full[I:, :] = ht.T.astype(NPBF16)

    WxT = np.asarray(Wx, dtype=np.float32).T   # [512, 3584]
    WhT = np.asarray(Wh, dtype=np.float32).T
    bsum = np.asarray(bx, dtype=np.float32) + np.asarray(bh, dtype=np.float32)
    w2 = np.empty((K2, G), dtype=NPBF16)
    bbp = np.empty(G, dtype=np.float32)
    for n, old in enumerate(PERM):
        dsl = slice(n * H, (n + 1) * H)
        ssl = slice(old * H, (old + 1) * H)
        w2[:I, dsl] = WxT[:, ssl].astype(NPBF16)
        w2[I:, dsl] = WhT[:, ssl].astype(NPBF16)
        bbp[dsl] = bsum[ssl]
    bb = np.ascontiguousarray(np.broadcast_to(bbp[None, :], (P, G)))

    in_maps = []
    for cidx in range(NCORES):
        sl = slice(cidx * BS, (cidx + 1) * BS)
        in_maps.append({
            "xh": np.ascontiguousarray(xh_full[:, sl]),
            "w2": w2,
            "ct": ct[sl],
            "bb": bb,
        })

    res = run_bass_kernel_spmd(nc, in_maps, core_ids=list(range(NCORES)),
                               trace=TRACE)
    LAST_RESULTS = res

    outs = {}
    for name in ("h", "c", "cb", "o", "dr"):
        outs[name] = np.concatenate(
            [res.results[cidx][name] for cidx in range(NCORES)], axis=0
        )
    return outs["h"], outs["c"], outs["cb"], outs["o"], outs["dr"]



# revision 3
# speedup vs baseline: 1.1990x; 1.1990x over previous
"""CTLSTM cell fused kernel for 8 Trainium2 NeuronCores (v2).

Strategy (data-parallel over batch, transposed weights-stationary GEMM):
  - B=16384 rows sharded 2048/core; weights replicated.
  - Compute the TRANSPOSED gate matrix per core: gatesT[3584, 2048] =
    W2T[K2, 3584].T @ xhT[K2, 2048] with the WEIGHT chunk as the PE
    stationary operand ([128,128] per (gate-chunk, k)) and xh as the
    moving operand (N=512 batch columns). Outputs stay transposed
    [H, batch] through SBUF and HBM; the host re-transposes and upcasts.
  - Host pre-stages everything in the exact consumption order:
      wcol[28][128, 8, 128]  bf16, gate-chunk-major (g' = j*7 + t with
        j = H/128 chunk, t in order [d, z, i, f, o, ib, fb]); one 256KB
        DMA per chunk so the PE can start after ~1MB instead of 11MB.
      xh4[4][128, 8, 512]    bf16, batch-supertile-major.
      ctT4[4][128, 2048]     bf16 transposed cell state.
      bb[128, 28]            f32 per-partition bias columns (bx+bh,
        negated for the d chunks so sigmoid(-(x+b)) = ACT(scale=-1,
        bias=-b)).
  - PSUM accumulates f32 over 8 k-chunks per (gate-chunk, bsup); the
    ACT engine drains PSUM directly with a FUSED per-partition bias +
    activation into bf16 SBUF stash (no DVE bias pass at all).
  - decay_rate = softplus(wd) = -ln(sigmoid(-wd)): the d gate is the
    FIRST chunk of each j so its Ln batch (different ACT table) runs
    early inside the j block; the two table switches per j hide under
    ~24us of matmul. No tail exposure.
  - Elementwise (c, h, c_bar) runs on DVE fully in bf16; all five
    outputs are written bf16 (halves output HBM traffic), host upcasts.
"""

import numpy as np
import ml_dtypes

import concourse.bacc as bacc
import concourse.bass as bass
import concourse.mybir as mybir
import concourse.tile as tile
from concourse.bass_utils import run_bass_kernel_spmd

NCORES = 8
B = 16384
I = 512
H = 512
NG = 7
G = NG * H          # 3584
K2 = I + H          # 1024
P = 128
BS = B // NCORES    # 2048 rows per core
NJ = H // P         # 4 H-chunks
NK = K2 // P        # 8 contraction chunks
NB = 4              # batch supertiles per core
BSUP = BS // NB     # 512
NGC = NG * NJ       # 28 gate chunks

BF16 = mybir.dt.bfloat16
F32 = mybir.dt.float32
AF = mybir.ActivationFunctionType
NPBF16 = ml_dtypes.bfloat16

# per-j gate-type order: d first (early Ln), then z, i, f, o, ib, fb.
# reference split order is i, f, z, o, d, i_bar, f_bar.
T_SRC = [4, 2, 0, 1, 3, 5, 6]   # t -> reference gate index
T_D, T_Z, T_I, T_F, T_O, T_IB, T_FB = range(7)

TRACE = False
LAST_RESULTS = None

_nc_cache = None


def _build():
    nc = bacc.Bacc("TRN2", target_bir_lowering=False, debug=False)

    w_d = nc.dram_tensor("w", [NGC, P, NK, P], BF16, kind="ExternalInput")
    xh_d = nc.dram_tensor("xh", [NB, P, NK, BSUP], BF16, kind="ExternalInput")
    ct_d = nc.dram_tensor("ct", [NJ, P, BS], BF16, kind="ExternalInput")
    bb_d = nc.dram_tensor("bb", [P, NGC], F32, kind="ExternalInput")

    h_d = nc.dram_tensor("h", [NJ, P, BS], BF16, kind="ExternalOutput")
    c_d = nc.dram_tensor("c", [NJ, P, BS], BF16, kind="ExternalOutput")
    cb_d = nc.dram_tensor("cb", [NJ, P, BS], BF16, kind="ExternalOutput")
    o_d = nc.dram_tensor("o", [NJ, P, BS], BF16, kind="ExternalOutput")
    dr_d = nc.dram_tensor("dr", [NJ, P, BS], BF16, kind="ExternalOutput")

    with tile.TileContext(nc) as tc:
        with (
            tc.tile_pool(name="wp", bufs=1) as wp,
            tc.tile_pool(name="xp", bufs=1) as xp,
            tc.tile_pool(name="ctp", bufs=1) as ctp,
            tc.tile_pool(name="bp", bufs=1) as bp,
            tc.tile_pool(name="sp", bufs=2) as sp,
            tc.tile_pool(name="Sp", bufs=2) as Sp,
            tc.tile_pool(name="op", bufs=1) as op,
            tc.tile_pool(name="tp", bufs=2) as tp,
            tc.tile_pool(name="pp", bufs=2, space=bass.MemorySpace.PSUM) as pp,
        ):
            # --- input DMAs, in PE consumption order -------------------
            w_sb = [None] * NGC
            xq = [None] * NB

            def dma_w(gp):
                w_sb[gp] = wp.tile([P, NK, P], BF16, tag=f"w{gp}", name=f"w{gp}")
                nc.sync.dma_start(w_sb[gp][:], w_d[gp])

            def dma_x(b):
                xq[b] = xp.tile([P, NK, BSUP], BF16, tag=f"x{b}", name=f"x{b}")
                nc.sync.dma_start(xq[b][:], xh_d[b])

            # j=0 needs w 0..6 and xh progressively; front-load those.
            dma_w(0)
            dma_x(0)
            dma_w(1)
            dma_x(1)
            dma_w(2)
            dma_x(2)
            dma_w(3)
            dma_x(3)
            bb = bp.tile([P, NGC], F32, tag="bb")
            nc.sync.dma_start(bb[:], bb_d[:])
            for gp in range(4, NGC):
                dma_w(gp)
            ct_sb = []
            for j in range(NJ):
                t_ = ctp.tile([P, BS], BF16, tag=f"ct{j}")
                nc.sync.dma_start(t_[:], ct_d[j])
                ct_sb.append(t_)

            # --- main loop --------------------------------------------
            for j in range(NJ):
                st = {}
                for t in (T_Z, T_I, T_F, T_IB, T_FB):
                    st[t] = sp.tile([P, BS], BF16, tag=f"s{t}", name=f"s{t}")
                S = Sp.tile([P, BS], F32, tag="S")
                out_h = op.tile([P, BS], BF16, tag="oh")
                out_c = op.tile([P, BS], BF16, tag="oc")
                out_cb = op.tile([P, BS], BF16, tag="ocb")
                out_o = op.tile([P, BS], BF16, tag="oo")
                out_dr = op.tile([P, BS], BF16, tag="odr")

                for t in range(NG):
                    gp = j * NG + t
                    bap = bb[:, gp:gp + 1]
                    for b in range(NB):
                        bsl = slice(b * BSUP, (b + 1) * BSUP)
                        acc = pp.tile([P, BSUP], F32, tag=f"a{b}")
                        for k in range(NK):
                            nc.tensor.matmul(
                                acc[:], w_sb[gp][:, k, :], xq[b][:, k, :],
                                start=(k == 0), stop=(k == NK - 1),
                            )
                        # fused bias + activation drain PSUM -> SBUF
                        if t == T_D:
                            # S = sigmoid(-(wd + b)) ; bb col holds -b
                            nc.scalar.activation(S[:, bsl], acc[:],
                                                 AF.Sigmoid, bias=bap,
                                                 scale=-1.0)
                        elif t == T_Z:
                            nc.scalar.activation(st[t][:, bsl], acc[:],
                                                 AF.Tanh, bias=bap)
                        elif t == T_O:
                            nc.scalar.activation(out_o[:, bsl], acc[:],
                                                 AF.Sigmoid, bias=bap)
                            nc.sync.dma_start(o_d[j][:, bsl], out_o[:, bsl])
                        else:
                            nc.scalar.activation(st[t][:, bsl], acc[:],
                                                 AF.Sigmoid, bias=bap)

                    if t == T_D:
                        # early softplus: dr = -ln(S); ACT switches to the
                        # ln table and back exactly once per j, hidden
                        # under the remaining 6 gate chunks of matmul.
                        nc.scalar.activation(S[:], S[:], AF.Ln)
                        nc.vector.tensor_scalar_mul(out_dr[:], S[:], -1.0)
                        nc.sync.dma_start(dr_d[j], out_dr[:])

                # elementwise per batch supertile, all bf16 on DVE
                for b in range(NB):
                    bsl = slice(b * BSUP, (b + 1) * BSUP)
                    ctj = ct_sb[j][:, bsl]
                    t1 = tp.tile([P, BSUP], BF16, tag="t1")
                    t2 = tp.tile([P, BSUP], BF16, tag="t2")
                    t3 = tp.tile([P, BSUP], BF16, tag="t3")
                    nc.vector.tensor_mul(t1[:], st[T_F][:, bsl], ctj)
                    nc.vector.tensor_mul(t2[:], st[T_I][:, bsl], st[T_Z][:, bsl])
                    nc.vector.tensor_add(out_c[:, bsl], t1[:], t2[:])
                    nc.sync.dma_start(c_d[j][:, bsl], out_c[:, bsl])
                    nc.scalar.activation(t3[:], out_c[:, bsl], AF.Tanh)
                    nc.vector.tensor_mul(out_h[:, bsl], out_o[:, bsl], t3[:])
                    nc.sync.dma_start(h_d[j][:, bsl], out_h[:, bsl])
                    nc.vector.tensor_mul(t1[:], st[T_FB][:, bsl], ctj)
                    nc.vector.tensor_mul(t2[:], st[T_IB][:, bsl], st[T_Z][:, bsl])
                    nc.vector.tensor_add(out_cb[:, bsl], t1[:], t2[:])
                    nc.sync.dma_start(cb_d[j][:, bsl], out_cb[:, bsl])

    nc.compile()
    return nc


def kernel(x, ht, ct, Wx, bx, Wh, bh):
    global _nc_cache, LAST_RESULTS
    if _nc_cache is None:
        _nc_cache = _build()
    nc = _nc_cache

    x = np.ascontiguousarray(x, dtype=np.float32)
    ht = np.ascontiguousarray(ht, dtype=np.float32)
    ct = np.ascontiguousarray(ct, dtype=np.float32)

    # ---- host staging ------------------------------------------------
    # xhT [K2, B] bf16
    xh_full = np.empty((K2, B), dtype=NPBF16)
    xh_full[:I, :] = x.T.astype(NPBF16)
    xh_full[I:, :] = ht.T.astype(NPBF16)

    WxT = np.asarray(Wx, dtype=np.float32).T   # [512, 3584]
    WhT = np.asarray(Wh, dtype=np.float32).T
    bsum = np.asarray(bx, dtype=np.float32) + np.asarray(bh, dtype=np.float32)

    # wcol [28][128, 8, 128] bf16 in g' = j*7 + t order; bb [128, 28] f32
    w2 = np.empty((K2, G), dtype=np.float32)
    w2[:I, :] = WxT
    w2[I:, :] = WhT
    wcol = np.empty((NGC, P, NK, P), dtype=NPBF16)
    bbp = np.empty((P, NGC), dtype=np.float32)
    for j in range(NJ):
        for t, src in enumerate(T_SRC):
            gp = j * NG + t
            csl = slice(src * H + j * P, src * H + (j + 1) * P)
            blk = w2[:, csl].reshape(NK, P, P)           # [k, p_part, m]
            wcol[gp] = blk.transpose(1, 0, 2).astype(NPBF16)
            sign = -1.0 if t == T_D else 1.0
            bbp[:, gp] = sign * bsum[csl]

    # ctT [512, B] bf16 -> per core [4, 128, 2048]
    ctT = np.ascontiguousarray(ct.T.astype(NPBF16))

    in_maps = []
    for cidx in range(NCORES):
        sl = slice(cidx * BS, (cidx + 1) * BS)
        xc = xh_full[:, sl].reshape(NK, P, NB, BSUP)
        in_maps.append({
            "w": wcol,
            "xh": np.ascontiguousarray(xc.transpose(2, 1, 0, 3)),
            "ct": np.ascontiguousarray(ctT[:, sl]).reshape(NJ, P, BS),
            "bb": bbp,
        })

    res = run_bass_kernel_spmd(nc, in_maps, core_ids=list(range(NCORES)),
                               trace=TRACE)
    LAST_RESULTS = res

    # ---- gather + un-transpose + upcast ------------------------------
    outs = {}
    for name in ("h", "c", "cb", "o", "dr"):
        full = np.empty((B, H), dtype=np.float32)
        for cidx in range(NCORES):
            r = np.asarray(res.results[cidx][name]).reshape(H, BS)
            full[cidx * BS:(cidx + 1) * BS, :] = r.T.astype(np.float32)
        outs[name] = full
    return outs["h"], outs["c"], outs["cb"], outs["o"], outs["dr"]
